# revision 19
# baseline (speedup 1.0000x reference)
"""BiLSTM-CRF Trainium2 kernel (8 NeuronCores, SPMD).

Strategy:
- Host: embedding gather E[sentence] -> per-core x^T slices (position-sharded;
  fwd LSTM on cores 0-3, bwd on cores 4-7; each core 1024 positions + 32-slot
  warmup pad).
- Device per core: X = x @ Wih^T + bias (split-fp16 matmuls), then chunked
  LSTM: 2 stream-groups x 32 sub-chunks x 16 steps with a 32-step warmup (the
  LSTM is contractive, so warm-started states converge to the true
  trajectory); fp16 recurrent weights / h stream, fp32 accumulation.
  Emissions via fp16 split matmul; AllGather of per-core partial emissions;
  chunked Viterbi forward scans: 4 streams per core in 32-partition blocks,
  2 DVE instructions per step (tensor_scalar with two per-partition scalars +
  transpose-reduce max).  fv for every position is stored.
- Host: backtrack from stored fv (warm-chunk offsets cancel in argmax);
  exact score via a chunk-overlap offset-correction chain.
"""

import sys

import numpy as np

try:
    import concourse.bass as bass
except Exception:  # pragma: no cover
    sys.path.insert(0, "/opt/trn_rl_repo")
    import concourse.bass as bass

from concourse import bacc

import concourse.mybir as mybir
from concourse.tile import TileContext

NTAGS, START_IX, STOP_IX, NEG = 15, 13, 14, -10000.0
D, HH = 256, 256
FP16 = mybir.dt.float16
F32 = mybir.dt.float32
SIG = mybir.ActivationFunctionType.Sigmoid
TANH = mybir.ActivationFunctionType.Tanh


class CFG:
    def __init__(self, T=4096, B=32):
        self.T = T
        self.NPC = T // 4             # positions per core (one direction)
        self.G = 2                    # stream groups per core
        self.B = B                    # sub-chunks per group
        self.W = 32                   # LSTM warmup steps (== slot pre-pad)
        self.L = self.NPC // (self.G * B)
        assert self.L * self.G * B == self.NPC
        self.S = self.W + self.L      # macro-steps per group
        self.NSLOT = self.NPC + 32
        self.VCH = max(32, T // 32)   # viterbi chunk length
        self.VW = 32                  # viterbi warmup
        self.VS = T // (8 * self.VCH)  # viterbi streams per core
        assert 1 <= self.VS <= 4
        self.VSTEP = self.VW + self.VCH
        self.VCOLS = self.VSTEP + 1


# ----------------------------------------------------------------------------
# device program
# ----------------------------------------------------------------------------

def build_program(cfg: CFG, debug=False):
    nc = bacc.Bacc("TRN2", target_bir_lowering=False, num_devices=8)
    C = cfg
    NS = C.NSLOT
    B = C.B
    add, mult = mybir.AluOpType.add, mybir.AluOpType.mult

    xT_in = nc.dram_tensor("xT_in", [128, 2 * NS], F32, kind="ExternalInput")
    whh_t = nc.dram_tensor("whh_t", [128, 2048], FP16, kind="ExternalInput")
    wih_hi = nc.dram_tensor("wih_hi", [128, 2048], FP16, kind="ExternalInput")
    wih_lo = nc.dram_tensor("wih_lo", [128, 2048], FP16, kind="ExternalInput")
    bias_in = nc.dram_tensor("bias_in", [128, 8], F32, kind="ExternalInput")
    wout_in = nc.dram_tensor("wout_in", [128, 4 * NTAGS], FP16, kind="ExternalInput")
    bout_in = nc.dram_tensor("bout_in", [128, 1], F32, kind="ExternalInput")
    tT_in = nc.dram_tensor("tT_in", [128, 32], F32, kind="ExternalInput")
    hmask_in = nc.dram_tensor("hmask_in", [128, 2], FP16, kind="ExternalInput")
    hinj_in = nc.dram_tensor("hinj_in", [128, 2], FP16, kind="ExternalInput")
    cmask_in = nc.dram_tensor("cmask_in", [128, 2], F32, kind="ExternalInput")
    cinj_in = nc.dram_tensor("cinj_in", [128, 2], F32, kind="ExternalInput")
    vmask_in = nc.dram_tensor("vmask_in", [128, 1], F32, kind="ExternalInput")
    vinit_in = nc.dram_tensor("vinit_in", [128, 1], F32, kind="ExternalInput")
    fmask_in = nc.dram_tensor("fmask_in", [128, 1], F32, kind="ExternalInput")
    esel_in = nc.dram_tensor("esel_in", [120, 3 * NTAGS], F32, kind="ExternalInput")
    pmask_in = nc.dram_tensor("pmask_in", [128, 2], F32, kind="ExternalInput")

    emis_local = nc.dram_tensor("emis_local", [NTAGS, NS], F32)
    emis_all = nc.dram_tensor("emis_all", [8 * NTAGS, NS], F32, addr_space="Shared")

    fvs_out = nc.dram_tensor("fvs_out", [128, C.VCOLS], F32, kind="ExternalOutput")
    feats_out = nc.dram_tensor("feats_out", [128, C.VCOLS], F32, kind="ExternalOutput")
    if debug:
        emis_dbg = nc.dram_tensor("emis_dbg", [NTAGS, NS], F32, kind="ExternalOutput")
        h_dbg = nc.dram_tensor("h_dbg", [128, 2 * B * (C.S + 1)], FP16, kind="ExternalOutput")
        x_dbg = nc.dram_tensor("x_dbg", [128, 8 * NS], F32, kind="ExternalOutput")

    with TileContext(nc) as tc:
        with (
            tc.tile_pool(name="const", bufs=1) as cpool,
            tc.tile_pool(name="big", bufs=1) as bpool,
            tc.tile_pool(name="scr", bufs=4) as spool,
            tc.tile_pool(name="psg", bufs=2, space="PSUM") as pspool,
            tc.tile_pool(name="psx", bufs=2, space="PSUM") as psxpool,
            tc.tile_pool(name="pse", bufs=2, space="PSUM") as psepool,
            tc.tile_pool(name="sac", bufs=1, space="PSUM") as sacpool,
        ):
            # ---- constants ----
            whh_sb = cpool.tile([128, 2, 8, 128], FP16, tag="whh")
            wih_hi_sb = cpool.tile([128, 2, 8, 128], FP16, tag="wihh")
            wih_lo_sb = cpool.tile([128, 2, 8, 128], FP16, tag="wihl")
            bias_sb = cpool.tile([128, 8], F32, tag="bias")
            wout_sb = cpool.tile([128, 2, 2, NTAGS], FP16, tag="wout")
            bout_sb = cpool.tile([128, 1], F32, tag="bout")
            tT_sb = cpool.tile([128, 32], F32, tag="tT")
            hmask_sb = cpool.tile([128, 2], FP16, tag="hmask")
            hinj_sb = cpool.tile([128, 2], FP16, tag="hinj")
            cmask_sb = cpool.tile([128, 2], F32, tag="cmask")
            cinj_sb = cpool.tile([128, 2], F32, tag="cinj")
            vmask_sb = cpool.tile([128, 1], F32, tag="vmask")
            vinit_sb = cpool.tile([128, 1], F32, tag="vinit")
            fmask_sb = cpool.tile([128, 1], F32, tag="fmask")
            esel_sb = cpool.tile([120, 3 * NTAGS], F32, tag="esel")
            pmask_sb = cpool.tile([128, 2], F32, tag="pmask")
            for sb, dr in [
                (bias_sb, bias_in), (bout_sb, bout_in), (tT_sb, tT_in),
                (hmask_sb, hmask_in), (hinj_sb, hinj_in), (cmask_sb, cmask_in),
                (cinj_sb, cinj_in), (vmask_sb, vmask_in), (vinit_sb, vinit_in),
                (fmask_sb, fmask_in), (esel_sb, esel_in), (pmask_sb, pmask_in),
            ]:
                nc.sync.dma_start(sb[:], dr[:])
            nc.sync.dma_start(whh_sb[:], whh_t[:].rearrange("p (a b c) -> p a b c", a=2, b=8))
            nc.sync.dma_start(wih_hi_sb[:], wih_hi[:].rearrange("p (a b c) -> p a b c", a=2, b=8))
            nc.sync.dma_start(wih_lo_sb[:], wih_lo[:].rearrange("p (a b c) -> p a b c", a=2, b=8))
            nc.sync.dma_start(wout_sb[:], wout_in[:].rearrange("p (a b c) -> p a b c", a=2, b=2))

            sac = sacpool.tile([1, 2], F32, name="sac")
            _pend = [None]
            obs_names = nc._obs_names = []
            stage_marks = nc._stage_marks = []

            def mark(stname):
                stage_marks.append((stname, nc.next_id()))

            def pe_observe(lhsT_ap, rhs_ap):
                # Sacrificial matmul: refreshes the PE engine's observed
                # clock for rhs_ap's producer so the following real matmuls
                # carry at most one wait each (the walrus MM encoding has a
                # single wait slot).
                _pend[0] = nc.tensor.matmul(sac[0:1, 0:1], lhsT_ap, rhs_ap,
                                            start=True, stop=True)
                obs_names.append(_pend[0].ins.name)

            def dep_mm(*args, **kw):
                ins = nc.tensor.matmul(*args, **kw)
                if _pend[0] is not None:
                    bass._add_dep_helper(ins.ins, _pend[0].ins, sync=True,
                                         reason="order after observer")
                return ins

            # ---- x^T ----
            xT_sb = bpool.tile([128, 2, NS], F32, tag="xT")
            nc.sync.dma_start(xT_sb[:], xT_in[:].rearrange("p (a b) -> p a b", a=2))
            xT16 = bpool.tile([128, 2, NS], FP16, tag="xT16")
            nc.vector.tensor_copy(xT16[:], xT_sb[:])

            mark("X")
            # ---- X = x @ Wih^T + bias : [128, 8, NS] ----
            X_sb = bpool.tile([128, 8, NS], F32, tag="X")
            pe_observe(xT16[:, 0, 0:1], xT16[:, 1, NS - 1:NS])
            pe_observe(tT_sb[:, 0:1], tT_sb[:, 0:1])
            xhist = []
            for n0 in range(0, NS, 512):
                nlen = min(512, NS - n0)
                for m in range(8):
                    ps = psxpool.tile([128, 512], F32, tag="psx")
                    if len(xhist) >= 2:
                        pm, pn = xhist[-2]
                        pe_observe(tT_sb[:, 0:1], X_sb[:, pm, pn:pn + 1])
                    xhist.append((m, n0))
                    step = 0
                    for wsb in (wih_hi_sb, wih_lo_sb):
                        for kt in range(2):
                            dep_mm(
                                ps[:, :nlen], wsb[:, kt, m, :],
                                xT16[:, kt, n0:n0 + nlen],
                                start=(step == 0), stop=(step == 3))
                            step += 1
                    nc.vector.tensor_scalar(
                        X_sb[:, m, n0:n0 + nlen], ps[:, :nlen],
                        bias_sb[:, m:m + 1], None, add)
            if debug:
                nc.sync.dma_start(x_dbg[:], X_sb[:].rearrange("p a b -> p (a b)"))

            mark("LSTM")
            # ---- LSTM ----
            h_all, c_st = [], []
            for g in range(C.G):
                hg = bpool.tile([128, 2, B, C.S + 1], FP16, tag=f"h{g}", name=f"h{g}")
                cg = bpool.tile([128, 2, B], F32, tag=f"c{g}", name=f"c{g}")
                h_all.append(hg)
                c_st.append(cg)
            for g in range(C.G):
                nc.vector.memset(h_all[g][:, :, :, 0], 0.0)
                nc.vector.memset(c_st[g][:], 0.0)

            half = C.NPC // C.G
            for k in range(C.S):
                for g in range(C.G):
                    if k == C.W and g == 0:
                        hsl = h_all[0][:, :, 0, C.W]
                        nc.vector.tensor_tensor(hsl, hsl, hmask_sb[:], op=mult)
                        nc.vector.tensor_tensor(hsl, hsl, hinj_sb[:], op=add)
                        csl = c_st[0][:, :, 0]
                        nc.vector.tensor_tensor(csl, csl, cmask_sb[:], op=mult)
                        nc.vector.tensor_tensor(csl, csl, cinj_sb[:], op=add)
                    ps = pspool.tile([128, 8 * B], F32, tag="psg")
                    pe_observe(hmask_sb[:, 0:1], h_all[g][:, 0, 0:1, k])
                    for m in range(8):
                        for kt in range(2):
                            dep_mm(
                                ps[:, m * B:(m + 1) * B],
                                whh_sb[:, kt, m, :],
                                h_all[g][:, kt, :, k],
                                start=(kt == 0), stop=(kt == 1))
                    u = spool.tile([128, 8, B], F32, tag="u")
                    st = half * g + k
                    xsl = X_sb[:, :, st: st + C.L * (B - 1) + 1: C.L]
                    nc.vector.tensor_tensor(
                        u[:], ps[:].rearrange("p (a b) -> p a b", a=8), xsl, op=add)
                    uf = u[:].rearrange("p a b -> p (a b)")
                    a = spool.tile([128, 6 * B], F32, tag="a")
                    gt = spool.tile([128, 2 * B], F32, tag="gt")
                    nc.scalar.activation(a[:], uf[:, :6 * B], SIG)
                    nc.scalar.activation(gt[:], uf[:, 6 * B:], TANH)
                    cf = c_st[g][:].rearrange("p a b -> p (a b)")
                    t1 = spool.tile([128, 2 * B], F32, tag="t1")
                    nc.vector.tensor_tensor(t1[:], a[:, :2 * B], gt[:], op=mult)
                    nc.vector.tensor_tensor(cf, a[:, 2 * B:4 * B], cf, op=mult)
                    nc.vector.tensor_tensor(cf, cf, t1[:], op=add)
                    tc2 = spool.tile([128, 2 * B], F32, tag="tc")
                    nc.scalar.activation(tc2[:], cf, TANH)
                    nc.vector.tensor_tensor(
                        h_all[g][:, :, :, k + 1].rearrange("p a b -> p (a b)"),
                        a[:, 4 * B:6 * B], tc2[:], op=mult)
            if debug:
                nc.sync.dma_start(h_dbg[:], h_all[0][:].rearrange("p a b c -> p (a b c)"))

            mark("EMIS")
            # ---- emissions -> emis_sb [NTAGS, NS] ----
            emis_sb = bpool.tile([NTAGS, NS], F32, tag="emis")
            epw = psepool.tile([NTAGS, 512], F32, tag="ep")
            pe_observe(hmask_sb[:, 0:1], h_all[0][:, 0, 0:1, C.S])
            step = 0
            for hl in range(2):
                for kt in range(2):
                    dep_mm(epw[:, :32], wout_sb[:, kt, hl, :],
                                     h_all[0][:, kt, 0, 1:33],
                                     start=(step == 0), stop=(step == 3))
                    step += 1
            nc.vector.tensor_scalar(emis_sb[:, 0:32], epw[:, :32],
                                    bout_sb[:NTAGS, :], None, add)
            lastcol = 0
            for g in range(C.G):
                for n0 in range(0, half, 512):
                    nlen = min(512, half - n0)
                    nb = nlen // C.L
                    b0 = n0 // C.L
                    ep = psepool.tile([NTAGS, 512], F32, tag="ep")
                    pe_observe(tT_sb[:NTAGS, 0:1], emis_sb[:, lastcol:lastcol + 1])
                    step = 0
                    for hl in range(2):
                        for kt in range(2):
                            hap = h_all[g][:, kt, b0:b0 + nb, C.W + 1:C.W + 1 + C.L]
                            dep_mm(ep[:, :nlen], wout_sb[:, kt, hl, :], hap,
                                             start=(step == 0), stop=(step == 3))
                            step += 1
                    nc.vector.tensor_scalar(
                        emis_sb[:, 32 + g * half + n0:32 + g * half + n0 + nlen],
                        ep[:, :nlen], bout_sb[:NTAGS, :], None, add)
                    lastcol = 32 + g * half + n0
            if debug:
                nc.sync.dma_start(emis_dbg[:], emis_sb[:])

            mark("GATHER")
            # ---- AllGather ----
            nc.sync.dma_start(emis_local[:], emis_sb[:])
            nc.gpsimd.collective_compute(
                "AllGather", mybir.AluOpType.bypass,
                replica_groups=[[0, 1, 2, 3, 4, 5, 6, 7]],
                ins=[emis_local[:].rearrange("a b -> (a b)")],
                outs=[emis_all[:].rearrange("a b -> (a b)")],
            )
            emis_full = bpool.tile([120, NS], F32, tag="efull")
            nc.sync.dma_start(emis_full[:], emis_all[:])

            # ---- selector matmuls: pick this core's Ef / Eb row-blocks ----
            efsel = bpool.tile([NTAGS, NS], F32, tag="efsel")
            ebsel = bpool.tile([NTAGS, NS], F32, tag="ebsel")
            pe_observe(tT_sb[:120, 0:1], emis_full[:, 0:1])
            shist = []
            for dst, scol in ((efsel, 0), (ebsel, NTAGS)):
                for n0 in range(0, NS, 512):
                    nlen = min(512, NS - n0)
                    ps = psxpool.tile([128, 512], F32, tag="psx")
                    if len(shist) >= 2:
                        pd, pn = shist[-2]
                        pe_observe(tT_sb[:NTAGS, 0:1], pd[:, pn:pn + 1])
                    shist.append((dst, n0))
                    dep_mm(ps[:NTAGS, :nlen],
                                     esel_sb[:, scol:scol + NTAGS],
                                     emis_full[:, n0:n0 + nlen],
                                     start=True, stop=True)
                    nc.vector.tensor_copy(dst[:, n0:n0 + nlen], ps[:NTAGS, :nlen])
            # third selector: previous bwd block, cols 32..63 (warmup Eb for
            # the (v2==0, s==0) stream); zero selector on other cores.
            eb2 = bpool.tile([NTAGS, 32], F32, tag="eb2")
            ps2 = psxpool.tile([128, 512], F32, tag="psx")
            pd, pn = shist[-2]
            pe_observe(tT_sb[:NTAGS, 0:1], pd[:, pn:pn + 1])
            dep_mm(ps2[:NTAGS, :32], esel_sb[:, 2 * NTAGS:],
                             emis_full[:, 32:64], start=True, stop=True)
            nc.vector.tensor_copy(eb2[:], ps2[:NTAGS, :32])

            mark("FEATS")
            # ---- feats assembly ----
            feats_arr = bpool.tile([128, C.VCOLS], F32, tag="feats")
            nc.vector.memset(feats_arr[:], 0.0)
            fstage = bpool.tile([NTAGS, C.VS, C.VSTEP], F32, tag="fstage")
            ebrev = ebsel[:, ::-1]
            for s in range(C.VS):
                dst = fstage[:, s, :]
                t0 = spool.tile([NTAGS, C.VSTEP], F32, tag="fb0")
                t1b = spool.tile([NTAGS, C.VSTEP], F32, tag="fb1")
                c0, c1 = C.VCH * s, half + C.VCH * s
                nc.vector.tensor_scalar(t0[:], efsel[:, c0:c0 + C.VSTEP],
                                        pmask_sb[:NTAGS, 0:1], None, mult)
                nc.vector.tensor_scalar(t1b[:], efsel[:, c1:c1 + C.VSTEP],
                                        pmask_sb[:NTAGS, 1:2], None, mult)
                nc.vector.tensor_tensor(dst, t0[:], t1b[:], op=add)
                if s >= 1:
                    e0 = spool.tile([NTAGS, C.VSTEP], F32, tag="fb0")
                    e1 = spool.tile([NTAGS, C.VSTEP], F32, tag="fb1")
                    nc.vector.tensor_scalar(e0[:], ebrev[:, c0 - 32:c0 + C.VCH],
                                            pmask_sb[:NTAGS, 0:1], None, mult)
                    nc.vector.tensor_scalar(e1[:], ebrev[:, c1 - 32:c1 + C.VCH],
                                            pmask_sb[:NTAGS, 1:2], None, mult)
                    nc.vector.tensor_tensor(e0[:], e0[:], e1[:], op=add)
                    nc.vector.tensor_tensor(dst, dst, e0[:], op=add)
                else:
                    e0 = spool.tile([NTAGS, C.VCH], F32, tag="fc0")
                    e1m = spool.tile([NTAGS, C.VCH], F32, tag="fc1")
                    nc.vector.tensor_scalar(e0[:], ebrev[:, c0:c0 + C.VCH],
                                            pmask_sb[:NTAGS, 0:1], None, mult)
                    nc.vector.tensor_scalar(e1m[:], ebrev[:, c1:c1 + C.VCH],
                                            pmask_sb[:NTAGS, 1:2], None, mult)
                    nc.vector.tensor_tensor(e0[:], e0[:], e1m[:], op=add)
                    dstm = fstage[:, s, C.VW:]
                    nc.vector.tensor_tensor(dstm, dstm, e0[:], op=add)
                    w1 = spool.tile([NTAGS, 32], F32, tag="wb1")
                    nc.vector.tensor_scalar(w1[:], ebrev[:, half - 32:half],
                                            pmask_sb[:NTAGS, 1:2], None, mult)
                    nc.vector.tensor_tensor(w1[:], w1[:], eb2[:, ::-1], op=add)
                    dstw = fstage[:, s, 0:32]
                    nc.vector.tensor_tensor(dstw, dstw, w1[:], op=add)
                nc.sync.dma_start(
                    feats_arr[32 * s:32 * s + NTAGS, 1:1 + C.VSTEP], fstage[:, s, :])
            nc.vector.tensor_scalar(
                feats_arr[:, 0:C.VW + 1], feats_arr[:, 0:C.VW + 1],
                fmask_sb[:], None, mult)

            mark("VITERBI")
            # ---- viterbi ----
            fvs = bpool.tile([128, C.VCOLS], F32, tag="fvs")
            nc.vector.memset(fvs[:, 0:1], 0.0)
            for k in range(C.VSTEP):
                if k == C.VW:
                    sl = fvs[:, k:k + 1]
                    nc.vector.tensor_tensor(sl, sl, vmask_sb[:], op=mult)
                    nc.vector.tensor_tensor(sl, sl, vinit_sb[:], op=add)
                vscr = spool.tile([128, 32], F32, tag="vscr")
                nc.vector.tensor_scalar(
                    vscr[:], tT_sb[:], fvs[:, k:k + 1], feats_arr[:, k:k + 1],
                    add, add)
                nc.vector.tensor_reduce(
                    fvs[:, k + 1:k + 2], vscr[:], op=mybir.AluOpType.max,
                    axis=mybir.AxisListType.X, apply_transpose=True)

            nc.sync.dma_start(fvs_out[:], fvs[:])
            nc.sync.dma_start(feats_out[:], feats_arr[:])

    nc.compile()
    return nc


# ----------------------------------------------------------------------------
# host prep / post
# ----------------------------------------------------------------------------

def _split16(W):
    hi = W.astype(np.float16)
    lo = (W.astype(np.float32) - hi.astype(np.float32)).astype(np.float16)
    return hi, lo


def prep_core_inputs(cfg, core, sentence, E, Wih, Whh, bih, bhh, W_out_half,
                     b_out, Tm, h0d, c0d):
    C = cfg
    T = C.T
    fwd = core < 4
    base = (core % 4) * C.NPC

    dpos = np.arange(C.NSLOT) + base - 32
    opos = dpos if fwd else (T - 1 - dpos)
    valid = (dpos >= 0) & (dpos < T)
    rows = np.zeros((C.NSLOT, D), np.float32)
    vi = np.where(valid)[0]
    rows[vi] = E[sentence[opos[vi]]]
    xT = rows.T.reshape(2, 128, C.NSLOT).transpose(1, 0, 2)

    def tiles(Wm):
        t = np.zeros((128, 2, 8, 128), np.float32)
        for kt in range(2):
            for m in range(8):
                t[:, kt, m, :] = Wm[m * 128:(m + 1) * 128, kt * 128:(kt + 1) * 128].T
        return t

    whh16 = tiles(Whh.astype(np.float16).astype(np.float32)).astype(np.float16)
    hi, lo = _split16(Wih)
    wih_hi16 = tiles(hi.astype(np.float32)).astype(np.float16)
    wih_lo16 = tiles(lo.astype(np.float32)).astype(np.float16)
    bias = (bih + bhh).astype(np.float32).reshape(8, 128).T.copy()

    wh, wl = _split16(W_out_half)
    wout = np.zeros((128, 2, 2, NTAGS), np.float16)
    for kt in range(2):
        wout[:, kt, 0, :] = wh[:, kt * 128:(kt + 1) * 128].T
        wout[:, kt, 1, :] = wl[:, kt * 128:(kt + 1) * 128].T

    bout = np.zeros((128, 1), np.float32)
    bout[:NTAGS, 0] = b_out.astype(np.float32) / 2.0

    tT = np.full((128, 32), NEG, np.float32)
    for s in range(4):
        tT[32 * s:32 * s + NTAGS, :NTAGS] = Tm.T

    inject = core in (0, 4)
    hmask = np.zeros((128, 2), np.float16) if inject else np.ones((128, 2), np.float16)
    cmask = np.zeros((128, 2), np.float32) if inject else np.ones((128, 2), np.float32)
    hinj = np.zeros((128, 2), np.float16)
    cinj = np.zeros((128, 2), np.float32)
    if inject:
        hinj[:, 0] = h0d[:128].astype(np.float16)
        hinj[:, 1] = h0d[128:].astype(np.float16)
        cinj[:, 0] = c0d[:128]
        cinj[:, 1] = c0d[128:]

    vmask = np.ones((128, 1), np.float32)
    vinit = np.zeros((128, 1), np.float32)
    fmask = np.ones((128, 1), np.float32)
    if core == 0:
        vmask[0:32] = 0.0
        vinit[0:32] = NEG
        vinit[START_IX] = 0.0
        fmask[0:32] = 0.0

    vh, v2 = core // 2, core % 2
    esel = np.zeros((120, 3 * NTAGS), np.float32)
    for tag in range(NTAGS):
        esel[vh * NTAGS + tag, tag] = 1.0
        esel[(7 - vh) * NTAGS + tag, NTAGS + tag] = 1.0
        if v2 == 0 and vh >= 1:
            esel[(8 - vh) * NTAGS + tag, 2 * NTAGS + tag] = 1.0
    pmask = np.zeros((128, 2), np.float32)
    pmask[:, 0] = 1.0 if v2 == 0 else 0.0
    pmask[:, 1] = 1.0 - pmask[:, 0]

    return {
        "xT_in": np.ascontiguousarray(xT.reshape(128, 2 * C.NSLOT), np.float32),
        "whh_t": np.ascontiguousarray(whh16.reshape(128, 2048)),
        "wih_hi": np.ascontiguousarray(wih_hi16.reshape(128, 2048)),
        "wih_lo": np.ascontiguousarray(wih_lo16.reshape(128, 2048)),
        "bias_in": bias,
        "wout_in": np.ascontiguousarray(wout.reshape(128, 4 * NTAGS)),
        "bout_in": bout,
        "tT_in": tT,
        "hmask_in": hmask, "hinj_in": hinj, "cmask_in": cmask, "cinj_in": cinj,
        "vmask_in": vmask, "vinit_in": vinit, "fmask_in": fmask,
        "esel_in": esel, "pmask_in": pmask,
    }


def host_finish(cfg, fvs_list, feats_list, Tm):
    C = cfg
    T = C.T
    NCH = T // C.VCH
    fv_main = np.zeros((T, NTAGS), np.float32)
    fv_warm_end = np.zeros((NCH, NTAGS), np.float32)
    feats = np.zeros((T, NTAGS), np.float32)
    for m in range(NCH):
        v, s = m // C.VS, m % C.VS
        blk = slice(32 * s, 32 * s + NTAGS)
        fv_main[m * C.VCH:(m + 1) * C.VCH] = fvs_list[v][blk, 1 + C.VW:].T
        fv_warm_end[m] = fvs_list[v][blk, C.VW]
        feats[m * C.VCH:(m + 1) * C.VCH] = feats_list[v][blk, 1 + C.VW:].T

    delta = np.float64(0.0)
    for m in range(1, NCH):
        p = m * C.VCH - 1
        a = fv_main[p]
        tag = int(np.argmax(a))
        delta = (np.float64(a[tag]) + delta) - np.float64(fv_warm_end[m][tag])
    fv_incl_last = fv_main[T - 1] + feats[T - 1]
    score = np.float32((fv_incl_last.astype(np.float64) + delta + Tm[STOP_IX]).max())

    path = np.zeros(T, np.int32)
    cur = int(np.argmax(fv_incl_last + Tm[STOP_IX]))
    path[T - 1] = cur
    for p in range(T - 1, 0, -1):
        cur = int(np.argmax(fv_main[p - 1] + feats[p - 1] + Tm[cur]))
        path[p - 1] = cur
    return score, path


def build_in_maps(cfg, sentence, E, W_ih_f, W_hh_f, b_ih_f, b_hh_f, W_ih_b,
                  W_hh_b, b_ih_b, b_hh_b, W_out, b_out, transitions, h0, c0):
    sentence = np.asarray(sentence).astype(np.int64)
    E = np.asarray(E, np.float32)
    Tm = np.asarray(transitions, np.float32)
    W_out = np.asarray(W_out, np.float32)
    b_out = np.asarray(b_out, np.float32)
    h0 = np.asarray(h0, np.float32)
    c0 = np.asarray(c0, np.float32)
    perm = np.r_[0:256, 256:512, 768:1024, 512:768]  # i,f,o,g row order

    in_maps = []
    for core in range(8):
        fwd = core < 4
        Wih = np.asarray(W_ih_f if fwd else W_ih_b, np.float32)[perm]
        Whh = np.asarray(W_hh_f if fwd else W_hh_b, np.float32)[perm]
        bih = np.asarray(b_ih_f if fwd else b_ih_b, np.float32)[perm]
        bhh = np.asarray(b_hh_f if fwd else b_hh_b, np.float32)[perm]
        Wh = W_out[:, :HH] if fwd else W_out[:, HH:]
        h0d = h0[0] if fwd else h0[1]
        c0d = c0[0] if fwd else c0[1]
        in_maps.append(prep_core_inputs(
            cfg, core, sentence, E, Wih, Whh, bih, bhh, Wh, b_out, Tm, h0d, c0d))
    return in_maps, Tm


_PROGRAM_CACHE = {}


def kernel(sentence, E, W_ih_f, W_hh_f, b_ih_f, b_hh_f, W_ih_b, W_hh_b,
           b_ih_b, b_hh_b, W_out, b_out, transitions, h0, c0):
    from concourse import bass_utils

    cfg = CFG()
    in_maps, Tm = build_in_maps(
        cfg, sentence, E, W_ih_f, W_hh_f, b_ih_f, b_hh_f, W_ih_b, W_hh_b,
        b_ih_b, b_hh_b, W_out, b_out, transitions, h0, c0)

    key = (cfg.T, cfg.B)
    if key not in _PROGRAM_CACHE:
        _PROGRAM_CACHE[key] = build_program(cfg)
    nc = _PROGRAM_CACHE[key]

    res = bass_utils.run_bass_kernel_spmd(nc, in_maps, core_ids=list(range(8)))
    fvs_list = [np.asarray(res.results[c]["fvs_out"]) for c in range(8)]
    feats_list = [np.asarray(res.results[c]["feats_out"]) for c in range(8)]
    score, path = host_finish(cfg, fvs_list, feats_list, Tm)
    return score, path


# revision 20
# speedup vs baseline: 1.1059x; 1.1059x over previous
"""BiLSTM-CRF Trainium2 kernel (8 NeuronCores, SPMD).

Strategy:
- Host: embedding gather E[sentence] -> per-core x^T slices (position-sharded;
  fwd LSTM on cores 0-3, bwd on cores 4-7; each core 1024 positions + 32-slot
  warmup pad).
- Device per core: X = x @ Wih^T + bias (split-fp16 matmuls), then chunked
  LSTM: 2 stream-groups x 32 sub-chunks x 16 steps with a 32-step warmup (the
  LSTM is contractive, so warm-started states converge to the true
  trajectory); fp16 recurrent weights / h stream, fp32 accumulation.
  Emissions via fp16 split matmul; AllGather of per-core partial emissions;
  chunked Viterbi forward scans: 4 streams per core in 32-partition blocks,
  2 DVE instructions per step (tensor_scalar with two per-partition scalars +
  transpose-reduce max).  fv for every position is stored.
- Host: backtrack from stored fv (warm-chunk offsets cancel in argmax);
  exact score via a chunk-overlap offset-correction chain.
"""

import sys

import numpy as np

try:
    import concourse.bass as bass
except Exception:  # pragma: no cover
    sys.path.insert(0, "/opt/trn_rl_repo")
    import concourse.bass as bass

from concourse import bacc

import concourse.mybir as mybir
from concourse.tile import TileContext

NTAGS, START_IX, STOP_IX, NEG = 15, 13, 14, -10000.0
D, HH = 256, 256
FP16 = mybir.dt.float16
F32 = mybir.dt.float32
SIG = mybir.ActivationFunctionType.Sigmoid
TANH = mybir.ActivationFunctionType.Tanh


class CFG:
    def __init__(self, T=4096, B=32):
        self.T = T
        self.NPC = T // 4             # positions per core (one direction)
        self.G = 2                    # stream groups per core
        self.B = B                    # sub-chunks per group
        self.W = 24                   # LSTM warmup steps (<= 32-slot pre-pad)
        self.L = self.NPC // (self.G * B)
        assert self.L * self.G * B == self.NPC
        self.S = self.W + self.L      # macro-steps per group
        self.NSLOT = self.NPC + 32
        self.VCH = max(32, T // 32)   # viterbi chunk length
        self.VW = 32                  # viterbi warmup
        self.VS = T // (8 * self.VCH)  # viterbi streams per core
        assert 1 <= self.VS <= 4
        self.VSTEP = self.VW + self.VCH
        self.VCOLS = self.VSTEP + 1


# ----------------------------------------------------------------------------
# device program
# ----------------------------------------------------------------------------

def build_program(cfg: CFG, debug=False):
    nc = bacc.Bacc("TRN2", target_bir_lowering=False, num_devices=8)
    C = cfg
    NS = C.NSLOT
    B = C.B
    add, mult = mybir.AluOpType.add, mybir.AluOpType.mult

    xT_in = nc.dram_tensor("xT_in", [128, 2 * NS], F32, kind="ExternalInput")
    whh_t = nc.dram_tensor("whh_t", [128, 2048], FP16, kind="ExternalInput")
    wih_hi = nc.dram_tensor("wih_hi", [128, 2048], FP16, kind="ExternalInput")
    wih_lo = nc.dram_tensor("wih_lo", [128, 2048], FP16, kind="ExternalInput")
    bias_in = nc.dram_tensor("bias_in", [128, 8], F32, kind="ExternalInput")
    wout_in = nc.dram_tensor("wout_in", [128, 4 * NTAGS], FP16, kind="ExternalInput")
    bout_in = nc.dram_tensor("bout_in", [128, 1], F32, kind="ExternalInput")
    tT_in = nc.dram_tensor("tT_in", [128, 32], F32, kind="ExternalInput")
    hmask_in = nc.dram_tensor("hmask_in", [128, 2], FP16, kind="ExternalInput")
    hinj_in = nc.dram_tensor("hinj_in", [128, 2], FP16, kind="ExternalInput")
    cmask_in = nc.dram_tensor("cmask_in", [128, 2], F32, kind="ExternalInput")
    cinj_in = nc.dram_tensor("cinj_in", [128, 2], F32, kind="ExternalInput")
    vmask_in = nc.dram_tensor("vmask_in", [128, 1], F32, kind="ExternalInput")
    vinit_in = nc.dram_tensor("vinit_in", [128, 1], F32, kind="ExternalInput")
    fmask_in = nc.dram_tensor("fmask_in", [128, 1], F32, kind="ExternalInput")
    esel_in = nc.dram_tensor("esel_in", [120, 3 * NTAGS], F32, kind="ExternalInput")
    pmask_in = nc.dram_tensor("pmask_in", [128, 2], F32, kind="ExternalInput")

    emis_local = nc.dram_tensor("emis_local", [NTAGS, NS], F32)
    emis_all = nc.dram_tensor("emis_all", [8 * NTAGS, NS], F32, addr_space="Shared")

    fvs_out = nc.dram_tensor("fvs_out", [128, C.VCOLS], F32, kind="ExternalOutput")
    feats_out = nc.dram_tensor("feats_out", [128, C.VCOLS], F32, kind="ExternalOutput")
    if debug:
        emis_dbg = nc.dram_tensor("emis_dbg", [NTAGS, NS], F32, kind="ExternalOutput")
        h_dbg = nc.dram_tensor("h_dbg", [128, 2 * B * (C.S + 1)], FP16, kind="ExternalOutput")
        x_dbg = nc.dram_tensor("x_dbg", [128, 8 * NS], F32, kind="ExternalOutput")

    with TileContext(nc) as tc:
        with (
            tc.tile_pool(name="const", bufs=1) as cpool,
            tc.tile_pool(name="big", bufs=1) as bpool,
            tc.tile_pool(name="scr", bufs=4) as spool,
            tc.tile_pool(name="psg", bufs=2, space="PSUM") as pspool,
            tc.tile_pool(name="psx", bufs=2, space="PSUM") as psxpool,
            tc.tile_pool(name="pse", bufs=2, space="PSUM") as psepool,
            tc.tile_pool(name="sac", bufs=1, space="PSUM") as sacpool,
        ):
            # ---- constants ----
            whh_sb = cpool.tile([128, 2, 8, 128], FP16, tag="whh")
            wih_hi_sb = cpool.tile([128, 2, 8, 128], FP16, tag="wihh")
            wih_lo_sb = cpool.tile([128, 2, 8, 128], FP16, tag="wihl")
            bias_sb = cpool.tile([128, 8], F32, tag="bias")
            wout_sb = cpool.tile([128, 2, 2, NTAGS], FP16, tag="wout")
            bout_sb = cpool.tile([128, 1], F32, tag="bout")
            tT_sb = cpool.tile([128, 32], F32, tag="tT")
            hmask_sb = cpool.tile([128, 2], FP16, tag="hmask")
            hinj_sb = cpool.tile([128, 2], FP16, tag="hinj")
            cmask_sb = cpool.tile([128, 2], F32, tag="cmask")
            cinj_sb = cpool.tile([128, 2], F32, tag="cinj")
            vmask_sb = cpool.tile([128, 1], F32, tag="vmask")
            vinit_sb = cpool.tile([128, 1], F32, tag="vinit")
            fmask_sb = cpool.tile([128, 1], F32, tag="fmask")
            esel_sb = cpool.tile([120, 3 * NTAGS], F32, tag="esel")
            pmask_sb = cpool.tile([128, 2], F32, tag="pmask")
            for sb, dr in [
                (bias_sb, bias_in), (bout_sb, bout_in), (tT_sb, tT_in),
                (hmask_sb, hmask_in), (hinj_sb, hinj_in), (cmask_sb, cmask_in),
                (cinj_sb, cinj_in), (vmask_sb, vmask_in), (vinit_sb, vinit_in),
                (fmask_sb, fmask_in), (esel_sb, esel_in), (pmask_sb, pmask_in),
            ]:
                nc.sync.dma_start(sb[:], dr[:])
            nc.sync.dma_start(whh_sb[:], whh_t[:].rearrange("p (a b c) -> p a b c", a=2, b=8))
            nc.sync.dma_start(wih_hi_sb[:], wih_hi[:].rearrange("p (a b c) -> p a b c", a=2, b=8))
            nc.sync.dma_start(wih_lo_sb[:], wih_lo[:].rearrange("p (a b c) -> p a b c", a=2, b=8))
            nc.sync.dma_start(wout_sb[:], wout_in[:].rearrange("p (a b c) -> p a b c", a=2, b=2))

            sac = sacpool.tile([1, 2], F32, name="sac")
            _pend = [None]
            obs_names = nc._obs_names = []
            stage_marks = nc._stage_marks = []

            def mark(stname):
                stage_marks.append((stname, nc.next_id()))

            def pe_observe(lhsT_ap, rhs_ap):
                # Sacrificial matmul: refreshes the PE engine's observed
                # clock for rhs_ap's producer so the following real matmuls
                # carry at most one wait each (the walrus MM encoding has a
                # single wait slot).
                _pend[0] = nc.tensor.matmul(sac[0:1, 0:1], lhsT_ap, rhs_ap,
                                            start=True, stop=True)
                obs_names.append(_pend[0].ins.name)

            def dep_mm(*args, **kw):
                ins = nc.tensor.matmul(*args, **kw)
                if _pend[0] is not None:
                    bass._add_dep_helper(ins.ins, _pend[0].ins, sync=True,
                                         reason="order after observer")
                return ins

            # ---- x^T (chunked load+cast so X matmuls start early) ----
            xT_sb = bpool.tile([128, 2, NS], F32, tag="xT")
            xT16 = bpool.tile([128, 2, NS], FP16, tag="xT16")
            xin3 = xT_in[:].rearrange("p (a b) -> p a b", a=2)
            for n0 in range(0, NS, 512):
                nlen = min(512, NS - n0)
                nc.sync.dma_start(xT_sb[:, :, n0:n0 + nlen], xin3[:, :, n0:n0 + nlen])
                nc.vector.tensor_copy(xT16[:, :, n0:n0 + nlen], xT_sb[:, :, n0:n0 + nlen])

            mark("X")
            # ---- X = x @ Wih^T + bias : [128, 8, NS] ----
            X_sb = bpool.tile([128, 8, NS], F32, tag="X")
            pe_observe(tT_sb[:, 0:1], tT_sb[:, 0:1])
            xhist = []
            for n0 in range(0, NS, 512):
                nlen = min(512, NS - n0)
                pe_observe(xT16[:, 0, n0:n0 + 1], xT16[:, 1, n0 + nlen - 1:n0 + nlen])
                for m in range(8):
                    ps = psxpool.tile([128, 512], F32, tag="psx")
                    if len(xhist) >= 2:
                        pm, pn = xhist[-2]
                        pe_observe(tT_sb[:, 0:1], X_sb[:, pm, pn:pn + 1])
                    xhist.append((m, n0))
                    step = 0
                    for wsb in (wih_hi_sb, wih_lo_sb):
                        for kt in range(2):
                            dep_mm(
                                ps[:, :nlen], wsb[:, kt, m, :],
                                xT16[:, kt, n0:n0 + nlen],
                                start=(step == 0), stop=(step == 3))
                            step += 1
                    nc.vector.tensor_scalar(
                        X_sb[:, m, n0:n0 + nlen], ps[:, :nlen],
                        bias_sb[:, m:m + 1], None, add)
            if debug:
                nc.sync.dma_start(x_dbg[:], X_sb[:].rearrange("p a b -> p (a b)"))

            mark("LSTM")
            # ---- LSTM ----
            h_all, c_st = [], []
            for g in range(C.G):
                hg = bpool.tile([128, 2, B, C.S + 1], FP16, tag=f"h{g}", name=f"h{g}")
                cg = bpool.tile([128, 2, B], F32, tag=f"c{g}", name=f"c{g}")
                h_all.append(hg)
                c_st.append(cg)
            for g in range(C.G):
                nc.vector.memset(h_all[g][:, :, :, 0], 0.0)
                nc.vector.memset(c_st[g][:], 0.0)

            half = C.NPC // C.G
            for k in range(C.S):
                for g in range(C.G):
                    if k == C.W and g == 0:
                        hsl = h_all[0][:, :, 0, C.W]
                        nc.vector.tensor_tensor(hsl, hsl, hmask_sb[:], op=mult)
                        nc.vector.tensor_tensor(hsl, hsl, hinj_sb[:], op=add)
                        csl = c_st[0][:, :, 0]
                        nc.vector.tensor_tensor(csl, csl, cmask_sb[:], op=mult)
                        nc.vector.tensor_tensor(csl, csl, cinj_sb[:], op=add)
                    ps = pspool.tile([128, 8 * B], F32, tag="psg")
                    pe_observe(hmask_sb[:, 0:1], h_all[g][:, 0, 0:1, k])
                    for m in range(8):
                        for kt in range(2):
                            dep_mm(
                                ps[:, m * B:(m + 1) * B],
                                whh_sb[:, kt, m, :],
                                h_all[g][:, kt, :, k],
                                start=(kt == 0), stop=(kt == 1))
                    u = spool.tile([128, 8, B], F32, tag="u")
                    st = half * g + k + (32 - C.W)
                    xsl = X_sb[:, :, st: st + C.L * (B - 1) + 1: C.L]
                    nc.vector.tensor_tensor(
                        u[:], ps[:].rearrange("p (a b) -> p a b", a=8), xsl, op=add)
                    uf = u[:].rearrange("p a b -> p (a b)")
                    a = spool.tile([128, 6 * B], F32, tag="a")
                    gt = spool.tile([128, 2 * B], F32, tag="gt")
                    nc.scalar.activation(a[:], uf[:, :6 * B], SIG)
                    nc.scalar.activation(gt[:], uf[:, 6 * B:], TANH)
                    cf = c_st[g][:].rearrange("p a b -> p (a b)")
                    t1 = spool.tile([128, 2 * B], F32, tag="t1")
                    nc.vector.tensor_tensor(t1[:], a[:, :2 * B], gt[:], op=mult)
                    nc.vector.tensor_tensor(cf, a[:, 2 * B:4 * B], cf, op=mult)
                    nc.vector.tensor_tensor(cf, cf, t1[:], op=add)
                    tc2 = spool.tile([128, 2 * B], F32, tag="tc")
                    nc.scalar.activation(tc2[:], cf, TANH)
                    nc.vector.tensor_tensor(
                        h_all[g][:, :, :, k + 1].rearrange("p a b -> p (a b)"),
                        a[:, 4 * B:6 * B], tc2[:], op=mult)
            if debug:
                nc.sync.dma_start(h_dbg[:], h_all[0][:].rearrange("p a b c -> p (a b c)"))

            mark("EMIS")
            # ---- emissions -> emis_sb [NTAGS, NS] ----
            emis_sb = bpool.tile([NTAGS, NS], F32, tag="emis")
            if C.W < 32:
                nc.vector.memset(emis_sb[:, 0:32 - C.W], 0.0)
            epw = psepool.tile([NTAGS, 512], F32, tag="ep")
            pe_observe(hmask_sb[:, 0:1], h_all[0][:, 0, 0:1, C.S])
            step = 0
            for hl in range(2):
                for kt in range(2):
                    dep_mm(epw[:, :C.W], wout_sb[:, kt, hl, :],
                           h_all[0][:, kt, 0, 1:C.W + 1],
                           start=(step == 0), stop=(step == 3))
                    step += 1
            nc.vector.tensor_scalar(emis_sb[:, 32 - C.W:32], epw[:, :C.W],
                                    bout_sb[:NTAGS, :], None, add)
            lastcol = 0
            for g in range(C.G):
                for n0 in range(0, half, 512):
                    nlen = min(512, half - n0)
                    nb = nlen // C.L
                    b0 = n0 // C.L
                    ep = psepool.tile([NTAGS, 512], F32, tag="ep")
                    pe_observe(tT_sb[:NTAGS, 0:1], emis_sb[:, lastcol:lastcol + 1])
                    step = 0
                    for hl in range(2):
                        for kt in range(2):
                            hap = h_all[g][:, kt, b0:b0 + nb, C.W + 1:C.W + 1 + C.L]
                            dep_mm(ep[:, :nlen], wout_sb[:, kt, hl, :], hap,
                                             start=(step == 0), stop=(step == 3))
                            step += 1
                    nc.vector.tensor_scalar(
                        emis_sb[:, 32 + g * half + n0:32 + g * half + n0 + nlen],
                        ep[:, :nlen], bout_sb[:NTAGS, :], None, add)
                    lastcol = 32 + g * half + n0
            if debug:
                nc.sync.dma_start(emis_dbg[:], emis_sb[:])

            mark("GATHER")
            # ---- AllGather ----
            nc.sync.dma_start(emis_local[:], emis_sb[:])
            nc.gpsimd.collective_compute(
                "AllGather", mybir.AluOpType.bypass,
                replica_groups=[[0, 1, 2, 3, 4, 5, 6, 7]],
                ins=[emis_local[:].rearrange("a b -> (a b)")],
                outs=[emis_all[:].rearrange("a b -> (a b)")],
            )
            emis_full = bpool.tile([120, NS], F32, tag="efull")
            nc.sync.dma_start(emis_full[:], emis_all[:])

            # ---- selector matmuls: pick this core's Ef / Eb row-blocks ----
            efsel = bpool.tile([NTAGS, NS], F32, tag="efsel")
            ebsel = bpool.tile([NTAGS, NS], F32, tag="ebsel")
            pe_observe(tT_sb[:120, 0:1], emis_full[:, 0:1])
            shist = []
            for dst, scol in ((efsel, 0), (ebsel, NTAGS)):
                for n0 in range(0, NS, 512):
                    nlen = min(512, NS - n0)
                    ps = psxpool.tile([128, 512], F32, tag="psx")
                    if len(shist) >= 2:
                        pd, pn = shist[-2]
                        pe_observe(tT_sb[:NTAGS, 0:1], pd[:, pn:pn + 1])
                    shist.append((dst, n0))
                    dep_mm(ps[:NTAGS, :nlen],
                                     esel_sb[:, scol:scol + NTAGS],
                                     emis_full[:, n0:n0 + nlen],
                                     start=True, stop=True)
                    nc.vector.tensor_copy(dst[:, n0:n0 + nlen], ps[:NTAGS, :nlen])
            # third selector: previous bwd block, cols 32..63 (warmup Eb for
            # the (v2==0, s==0) stream); zero selector on other cores.
            eb2 = bpool.tile([NTAGS, 32], F32, tag="eb2")
            ps2 = psxpool.tile([128, 512], F32, tag="psx")
            pd, pn = shist[-2]
            pe_observe(tT_sb[:NTAGS, 0:1], pd[:, pn:pn + 1])
            dep_mm(ps2[:NTAGS, :32], esel_sb[:, 2 * NTAGS:],
                             emis_full[:, 32:64], start=True, stop=True)
            nc.vector.tensor_copy(eb2[:], ps2[:NTAGS, :32])

            mark("FEATS")
            # ---- feats assembly ----
            feats_arr = bpool.tile([128, C.VCOLS], F32, tag="feats")
            nc.vector.memset(feats_arr[:], 0.0)
            fstage = bpool.tile([NTAGS, C.VS, C.VSTEP], F32, tag="fstage")
            ebrev = ebsel[:, ::-1]
            for s in range(C.VS):
                dst = fstage[:, s, :]
                t0 = spool.tile([NTAGS, C.VSTEP], F32, tag="fb0")
                t1b = spool.tile([NTAGS, C.VSTEP], F32, tag="fb1")
                c0, c1 = C.VCH * s, half + C.VCH * s
                nc.vector.tensor_scalar(t0[:], efsel[:, c0:c0 + C.VSTEP],
                                        pmask_sb[:NTAGS, 0:1], None, mult)
                nc.vector.tensor_scalar(t1b[:], efsel[:, c1:c1 + C.VSTEP],
                                        pmask_sb[:NTAGS, 1:2], None, mult)
                nc.vector.tensor_tensor(dst, t0[:], t1b[:], op=add)
                if s >= 1:
                    e0 = spool.tile([NTAGS, C.VSTEP], F32, tag="fb0")
                    e1 = spool.tile([NTAGS, C.VSTEP], F32, tag="fb1")
                    nc.vector.tensor_scalar(e0[:], ebrev[:, c0 - 32:c0 + C.VCH],
                                            pmask_sb[:NTAGS, 0:1], None, mult)
                    nc.vector.tensor_scalar(e1[:], ebrev[:, c1 - 32:c1 + C.VCH],
                                            pmask_sb[:NTAGS, 1:2], None, mult)
                    nc.vector.tensor_tensor(e0[:], e0[:], e1[:], op=add)
                    nc.vector.tensor_tensor(dst, dst, e0[:], op=add)
                else:
                    e0 = spool.tile([NTAGS, C.VCH], F32, tag="fc0")
                    e1m = spool.tile([NTAGS, C.VCH], F32, tag="fc1")
                    nc.vector.tensor_scalar(e0[:], ebrev[:, c0:c0 + C.VCH],
                                            pmask_sb[:NTAGS, 0:1], None, mult)
                    nc.vector.tensor_scalar(e1m[:], ebrev[:, c1:c1 + C.VCH],
                                            pmask_sb[:NTAGS, 1:2], None, mult)
                    nc.vector.tensor_tensor(e0[:], e0[:], e1m[:], op=add)
                    dstm = fstage[:, s, C.VW:]
                    nc.vector.tensor_tensor(dstm, dstm, e0[:], op=add)
                    w1 = spool.tile([NTAGS, 32], F32, tag="wb1")
                    nc.vector.tensor_scalar(w1[:], ebrev[:, half - 32:half],
                                            pmask_sb[:NTAGS, 1:2], None, mult)
                    nc.vector.tensor_tensor(w1[:], w1[:], eb2[:, ::-1], op=add)
                    dstw = fstage[:, s, 0:32]
                    nc.vector.tensor_tensor(dstw, dstw, w1[:], op=add)
                nc.sync.dma_start(
                    feats_arr[32 * s:32 * s + NTAGS, 1:1 + C.VSTEP], fstage[:, s, :])
            nc.vector.tensor_scalar(
                feats_arr[:, 0:C.VW + 1], feats_arr[:, 0:C.VW + 1],
                fmask_sb[:], None, mult)

            mark("VITERBI")
            # ---- viterbi ----
            fvs = bpool.tile([128, C.VCOLS], F32, tag="fvs")
            nc.vector.memset(fvs[:, 0:1], 0.0)
            for k in range(C.VSTEP):
                if k == C.VW:
                    sl = fvs[:, k:k + 1]
                    nc.vector.tensor_tensor(sl, sl, vmask_sb[:], op=mult)
                    nc.vector.tensor_tensor(sl, sl, vinit_sb[:], op=add)
                vscr = spool.tile([128, 32], F32, tag="vscr")
                nc.vector.tensor_scalar(
                    vscr[:], tT_sb[:], fvs[:, k:k + 1], feats_arr[:, k:k + 1],
                    add, add)
                nc.vector.tensor_reduce(
                    fvs[:, k + 1:k + 2], vscr[:], op=mybir.AluOpType.max,
                    axis=mybir.AxisListType.X, apply_transpose=True)

            nc.sync.dma_start(fvs_out[:], fvs[:])
            nc.sync.dma_start(feats_out[:], feats_arr[:])

    nc.compile()
    return nc


# ----------------------------------------------------------------------------
# host prep / post
# ----------------------------------------------------------------------------

def _split16(W):
    hi = W.astype(np.float16)
    lo = (W.astype(np.float32) - hi.astype(np.float32)).astype(np.float16)
    return hi, lo


def prep_core_inputs(cfg, core, sentence, E, Wih, Whh, bih, bhh, W_out_half,
                     b_out, Tm, h0d, c0d):
    C = cfg
    T = C.T
    fwd = core < 4
    base = (core % 4) * C.NPC

    dpos = np.arange(C.NSLOT) + base - 32
    opos = dpos if fwd else (T - 1 - dpos)
    valid = (dpos >= 0) & (dpos < T)
    rows = np.zeros((C.NSLOT, D), np.float32)
    vi = np.where(valid)[0]
    rows[vi] = E[sentence[opos[vi]]]
    xT = rows.T.reshape(2, 128, C.NSLOT).transpose(1, 0, 2)

    def tiles(Wm):
        t = np.zeros((128, 2, 8, 128), np.float32)
        for kt in range(2):
            for m in range(8):
                t[:, kt, m, :] = Wm[m * 128:(m + 1) * 128, kt * 128:(kt + 1) * 128].T
        return t

    whh16 = tiles(Whh.astype(np.float16).astype(np.float32)).astype(np.float16)
    hi, lo = _split16(Wih)
    wih_hi16 = tiles(hi.astype(np.float32)).astype(np.float16)
    wih_lo16 = tiles(lo.astype(np.float32)).astype(np.float16)
    bias = (bih + bhh).astype(np.float32).reshape(8, 128).T.copy()

    wh, wl = _split16(W_out_half)
    wout = np.zeros((128, 2, 2, NTAGS), np.float16)
    for kt in range(2):
        wout[:, kt, 0, :] = wh[:, kt * 128:(kt + 1) * 128].T
        wout[:, kt, 1, :] = wl[:, kt * 128:(kt + 1) * 128].T

    bout = np.zeros((128, 1), np.float32)
    bout[:NTAGS, 0] = b_out.astype(np.float32) / 2.0

    tT = np.full((128, 32), NEG, np.float32)
    for s in range(4):
        tT[32 * s:32 * s + NTAGS, :NTAGS] = Tm.T

    inject = core in (0, 4)
    hmask = np.zeros((128, 2), np.float16) if inject else np.ones((128, 2), np.float16)
    cmask = np.zeros((128, 2), np.float32) if inject else np.ones((128, 2), np.float32)
    hinj = np.zeros((128, 2), np.float16)
    cinj = np.zeros((128, 2), np.float32)
    if inject:
        hinj[:, 0] = h0d[:128].astype(np.float16)
        hinj[:, 1] = h0d[128:].astype(np.float16)
        cinj[:, 0] = c0d[:128]
        cinj[:, 1] = c0d[128:]

    vmask = np.ones((128, 1), np.float32)
    vinit = np.zeros((128, 1), np.float32)
    fmask = np.ones((128, 1), np.float32)
    if core == 0:
        vmask[0:32] = 0.0
        vinit[0:32] = NEG
        vinit[START_IX] = 0.0
        fmask[0:32] = 0.0

    vh, v2 = core // 2, core % 2
    esel = np.zeros((120, 3 * NTAGS), np.float32)
    for tag in range(NTAGS):
        esel[vh * NTAGS + tag, tag] = 1.0
        esel[(7 - vh) * NTAGS + tag, NTAGS + tag] = 1.0
        if v2 == 0 and vh >= 1:
            esel[(8 - vh) * NTAGS + tag, 2 * NTAGS + tag] = 1.0
    pmask = np.zeros((128, 2), np.float32)
    pmask[:, 0] = 1.0 if v2 == 0 else 0.0
    pmask[:, 1] = 1.0 - pmask[:, 0]

    return {
        "xT_in": np.ascontiguousarray(xT.reshape(128, 2 * C.NSLOT), np.float32),
        "whh_t": np.ascontiguousarray(whh16.reshape(128, 2048)),
        "wih_hi": np.ascontiguousarray(wih_hi16.reshape(128, 2048)),
        "wih_lo": np.ascontiguousarray(wih_lo16.reshape(128, 2048)),
        "bias_in": bias,
        "wout_in": np.ascontiguousarray(wout.reshape(128, 4 * NTAGS)),
        "bout_in": bout,
        "tT_in": tT,
        "hmask_in": hmask, "hinj_in": hinj, "cmask_in": cmask, "cinj_in": cinj,
        "vmask_in": vmask, "vinit_in": vinit, "fmask_in": fmask,
        "esel_in": esel, "pmask_in": pmask,
    }


def host_finish(cfg, fvs_list, feats_list, Tm):
    C = cfg
    T = C.T
    NCH = T // C.VCH
    fv_main = np.zeros((T, NTAGS), np.float32)
    fv_warm_end = np.zeros((NCH, NTAGS), np.float32)
    feats = np.zeros((T, NTAGS), np.float32)
    for m in range(NCH):
        v, s = m // C.VS, m % C.VS
        blk = slice(32 * s, 32 * s + NTAGS)
        fv_main[m * C.VCH:(m + 1) * C.VCH] = fvs_list[v][blk, 1 + C.VW:].T
        fv_warm_end[m] = fvs_list[v][blk, C.VW]
        feats[m * C.VCH:(m + 1) * C.VCH] = feats_list[v][blk, 1 + C.VW:].T

    delta = np.float64(0.0)
    for m in range(1, NCH):
        p = m * C.VCH - 1
        a = fv_main[p]
        tag = int(np.argmax(a))
        delta = (np.float64(a[tag]) + delta) - np.float64(fv_warm_end[m][tag])
    fv_incl_last = fv_main[T - 1] + feats[T - 1]
    score = np.float32((fv_incl_last.astype(np.float64) + delta + Tm[STOP_IX]).max())

    path = np.zeros(T, np.int32)
    cur = int(np.argmax(fv_incl_last + Tm[STOP_IX]))
    path[T - 1] = cur
    for p in range(T - 1, 0, -1):
        cur = int(np.argmax(fv_main[p - 1] + feats[p - 1] + Tm[cur]))
        path[p - 1] = cur
    return score, path


def build_in_maps(cfg, sentence, E, W_ih_f, W_hh_f, b_ih_f, b_hh_f, W_ih_b,
                  W_hh_b, b_ih_b, b_hh_b, W_out, b_out, transitions, h0, c0):
    sentence = np.asarray(sentence).astype(np.int64)
    E = np.asarray(E, np.float32)
    Tm = np.asarray(transitions, np.float32)
    W_out = np.asarray(W_out, np.float32)
    b_out = np.asarray(b_out, np.float32)
    h0 = np.asarray(h0, np.float32)
    c0 = np.asarray(c0, np.float32)
    perm = np.r_[0:256, 256:512, 768:1024, 512:768]  # i,f,o,g row order

    in_maps = []
    for core in range(8):
        fwd = core < 4
        Wih = np.asarray(W_ih_f if fwd else W_ih_b, np.float32)[perm]
        Whh = np.asarray(W_hh_f if fwd else W_hh_b, np.float32)[perm]
        bih = np.asarray(b_ih_f if fwd else b_ih_b, np.float32)[perm]
        bhh = np.asarray(b_hh_f if fwd else b_hh_b, np.float32)[perm]
        Wh = W_out[:, :HH] if fwd else W_out[:, HH:]
        h0d = h0[0] if fwd else h0[1]
        c0d = c0[0] if fwd else c0[1]
        in_maps.append(prep_core_inputs(
            cfg, core, sentence, E, Wih, Whh, bih, bhh, Wh, b_out, Tm, h0d, c0d))
    return in_maps, Tm


_PROGRAM_CACHE = {}


def kernel(sentence, E, W_ih_f, W_hh_f, b_ih_f, b_hh_f, W_ih_b, W_hh_b,
           b_ih_b, b_hh_b, W_out, b_out, transitions, h0, c0):
    from concourse import bass_utils

    cfg = CFG()
    in_maps, Tm = build_in_maps(
        cfg, sentence, E, W_ih_f, W_hh_f, b_ih_f, b_hh_f, W_ih_b, W_hh_b,
        b_ih_b, b_hh_b, W_out, b_out, transitions, h0, c0)

    key = (cfg.T, cfg.B)
    if key not in _PROGRAM_CACHE:
        _PROGRAM_CACHE[key] = build_program(cfg)
    nc = _PROGRAM_CACHE[key]

    res = bass_utils.run_bass_kernel_spmd(nc, in_maps, core_ids=list(range(8)))
    fvs_list = [np.asarray(res.results[c]["fvs_out"]) for c in range(8)]
    feats_list = [np.asarray(res.results[c]["feats_out"]) for c in range(8)]
    score, path = host_finish(cfg, fvs_list, feats_list, Tm)
    return score, path


# revision 22
# speedup vs baseline: 1.1617x; 1.0505x over previous
"""BiLSTM-CRF Trainium2 kernel (8 NeuronCores, SPMD).

Strategy:
- Host: embedding gather E[sentence] -> per-core x^T slices (position-sharded;
  fwd LSTM on cores 0-3, bwd on cores 4-7; each core 1024 positions + 32-slot
  warmup pad).
- Device per core: X = x @ Wih^T + bias (split-fp16 matmuls), then chunked
  LSTM: 2 stream-groups x 32 sub-chunks x 16 steps with a 32-step warmup (the
  LSTM is contractive, so warm-started states converge to the true
  trajectory); fp16 recurrent weights / h stream, fp32 accumulation.
  Emissions via fp16 split matmul; AllGather of per-core partial emissions;
  chunked Viterbi forward scans: 4 streams per core in 32-partition blocks,
  2 DVE instructions per step (tensor_scalar with two per-partition scalars +
  transpose-reduce max).  fv for every position is stored.
- Host: backtrack from stored fv (warm-chunk offsets cancel in argmax);
  exact score via a chunk-overlap offset-correction chain.
"""

import sys

import numpy as np

try:
    import concourse.bass as bass
except Exception:  # pragma: no cover
    sys.path.insert(0, "/opt/trn_rl_repo")
    import concourse.bass as bass

from concourse import bacc

import concourse.mybir as mybir
from concourse.tile import TileContext

NTAGS, START_IX, STOP_IX, NEG = 15, 13, 14, -10000.0
D, HH = 256, 256
FP16 = mybir.dt.float16
F32 = mybir.dt.float32
SIG = mybir.ActivationFunctionType.Sigmoid
TANH = mybir.ActivationFunctionType.Tanh


class CFG:
    def __init__(self, T=4096, B=32):
        self.T = T
        self.NPC = T // 4             # positions per core (one direction)
        self.G = 2                    # stream groups per core
        self.B = B                    # sub-chunks per group
        self.W = 24                   # LSTM warmup steps (<= 32-slot pre-pad)
        self.L = self.NPC // (self.G * B)
        assert self.L * self.G * B == self.NPC
        self.S = self.W + self.L      # macro-steps per group
        self.NSLOT = self.NPC + 32
        self.VCH = max(32, T // 32)   # viterbi chunk length
        self.VW = 32                  # viterbi warmup
        self.VS = T // (8 * self.VCH)  # viterbi streams per core
        assert 1 <= self.VS <= 4
        self.VSTEP = self.VW + self.VCH
        self.VCOLS = self.VSTEP + 1


# ----------------------------------------------------------------------------
# device program
# ----------------------------------------------------------------------------

def build_program(cfg: CFG, debug=False):
    nc = bacc.Bacc("TRN2", target_bir_lowering=False, num_devices=8)
    C = cfg
    NS = C.NSLOT
    B = C.B
    add, mult = mybir.AluOpType.add, mybir.AluOpType.mult

    xT_in = nc.dram_tensor("xT_in", [128, 2 * NS], F32, kind="ExternalInput")
    whh_t = nc.dram_tensor("whh_t", [128, 2048], FP16, kind="ExternalInput")
    wih_hi = nc.dram_tensor("wih_hi", [128, 2048], FP16, kind="ExternalInput")
    wih_lo = nc.dram_tensor("wih_lo", [128, 2048], FP16, kind="ExternalInput")
    bias_in = nc.dram_tensor("bias_in", [128, 8], F32, kind="ExternalInput")
    wout_in = nc.dram_tensor("wout_in", [128, 4 * NTAGS], FP16, kind="ExternalInput")
    bout_in = nc.dram_tensor("bout_in", [128, 1], F32, kind="ExternalInput")
    tT_in = nc.dram_tensor("tT_in", [128, 32], F32, kind="ExternalInput")
    hmask_in = nc.dram_tensor("hmask_in", [128, 2], FP16, kind="ExternalInput")
    hinj_in = nc.dram_tensor("hinj_in", [128, 2], FP16, kind="ExternalInput")
    cmask_in = nc.dram_tensor("cmask_in", [128, 2], F32, kind="ExternalInput")
    cinj_in = nc.dram_tensor("cinj_in", [128, 2], F32, kind="ExternalInput")
    vmask_in = nc.dram_tensor("vmask_in", [128, 1], F32, kind="ExternalInput")
    vinit_in = nc.dram_tensor("vinit_in", [128, 1], F32, kind="ExternalInput")
    fmask_in = nc.dram_tensor("fmask_in", [128, 1], F32, kind="ExternalInput")
    esel_in = nc.dram_tensor("esel_in", [120, 3 * NTAGS], FP16, kind="ExternalInput")
    pmask_in = nc.dram_tensor("pmask_in", [128, 2], F32, kind="ExternalInput")

    emis_local = nc.dram_tensor("emis_local", [NTAGS, NS], FP16)
    emis_all = nc.dram_tensor("emis_all", [8 * NTAGS, NS], FP16, addr_space="Shared")

    fvs_out = nc.dram_tensor("fvs_out", [128, C.VCOLS], F32, kind="ExternalOutput")
    feats_out = nc.dram_tensor("feats_out", [128, C.VCOLS], F32, kind="ExternalOutput")
    if debug:
        emis_dbg = nc.dram_tensor("emis_dbg", [NTAGS, NS], FP16, kind="ExternalOutput")
        h_dbg = nc.dram_tensor("h_dbg", [128, 2 * B * (C.S + 1)], FP16, kind="ExternalOutput")
        x_dbg = nc.dram_tensor("x_dbg", [128, 8 * NS], F32, kind="ExternalOutput")

    with TileContext(nc) as tc:
        with (
            tc.tile_pool(name="const", bufs=1) as cpool,
            tc.tile_pool(name="big", bufs=1) as bpool,
            tc.tile_pool(name="scr", bufs=4) as spool,
            tc.tile_pool(name="psg", bufs=2, space="PSUM") as pspool,
            tc.tile_pool(name="psx", bufs=2, space="PSUM") as psxpool,
            tc.tile_pool(name="pse", bufs=2, space="PSUM") as psepool,
            tc.tile_pool(name="sac", bufs=1, space="PSUM") as sacpool,
        ):
            # ---- constants ----
            whh_sb = cpool.tile([128, 2, 8, 128], FP16, tag="whh")
            wih_hi_sb = cpool.tile([128, 2, 8, 128], FP16, tag="wihh")
            wih_lo_sb = cpool.tile([128, 2, 8, 128], FP16, tag="wihl")
            bias_sb = cpool.tile([128, 8], F32, tag="bias")
            wout_sb = cpool.tile([128, 2, 2, NTAGS], FP16, tag="wout")
            bout_sb = cpool.tile([128, 1], F32, tag="bout")
            tT_sb = cpool.tile([128, 32], F32, tag="tT")
            hmask_sb = cpool.tile([128, 2], FP16, tag="hmask")
            hinj_sb = cpool.tile([128, 2], FP16, tag="hinj")
            cmask_sb = cpool.tile([128, 2], F32, tag="cmask")
            cinj_sb = cpool.tile([128, 2], F32, tag="cinj")
            vmask_sb = cpool.tile([128, 1], F32, tag="vmask")
            vinit_sb = cpool.tile([128, 1], F32, tag="vinit")
            fmask_sb = cpool.tile([128, 1], F32, tag="fmask")
            esel_sb = cpool.tile([120, 3 * NTAGS], FP16, tag="esel")
            pmask_sb = cpool.tile([128, 2], F32, tag="pmask")
            for sb, dr in [
                (bias_sb, bias_in), (bout_sb, bout_in), (tT_sb, tT_in),
                (hmask_sb, hmask_in), (hinj_sb, hinj_in), (cmask_sb, cmask_in),
                (cinj_sb, cinj_in), (vmask_sb, vmask_in), (vinit_sb, vinit_in),
                (fmask_sb, fmask_in), (esel_sb, esel_in), (pmask_sb, pmask_in),
            ]:
                nc.sync.dma_start(sb[:], dr[:])
            nc.sync.dma_start(whh_sb[:], whh_t[:].rearrange("p (a b c) -> p a b c", a=2, b=8))
            nc.sync.dma_start(wih_hi_sb[:], wih_hi[:].rearrange("p (a b c) -> p a b c", a=2, b=8))
            nc.sync.dma_start(wih_lo_sb[:], wih_lo[:].rearrange("p (a b c) -> p a b c", a=2, b=8))
            nc.sync.dma_start(wout_sb[:], wout_in[:].rearrange("p (a b c) -> p a b c", a=2, b=2))

            sac = sacpool.tile([1, 2], F32, name="sac")
            _pend = [None]
            obs_names = nc._obs_names = []
            stage_marks = nc._stage_marks = []

            def mark(stname):
                stage_marks.append((stname, nc.next_id()))

            def pe_observe(lhsT_ap, rhs_ap):
                # Sacrificial matmul: refreshes the PE engine's observed
                # clock for rhs_ap's producer so the following real matmuls
                # carry at most one wait each (the walrus MM encoding has a
                # single wait slot).
                _pend[0] = nc.tensor.matmul(sac[0:1, 0:1], lhsT_ap, rhs_ap,
                                            start=True, stop=True)
                obs_names.append(_pend[0].ins.name)

            def dep_mm(*args, **kw):
                ins = nc.tensor.matmul(*args, **kw)
                if _pend[0] is not None:
                    bass._add_dep_helper(ins.ins, _pend[0].ins, sync=True,
                                         reason="order after observer")
                return ins

            # ---- x^T (chunked load+cast so X matmuls start early) ----
            xT_sb = bpool.tile([128, 2, NS], F32, tag="xT")
            xT16 = bpool.tile([128, 2, NS], FP16, tag="xT16")
            xin3 = xT_in[:].rearrange("p (a b) -> p a b", a=2)
            for n0 in range(0, NS, 512):
                nlen = min(512, NS - n0)
                nc.sync.dma_start(xT_sb[:, :, n0:n0 + nlen], xin3[:, :, n0:n0 + nlen])
                nc.vector.tensor_copy(xT16[:, :, n0:n0 + nlen], xT_sb[:, :, n0:n0 + nlen])

            mark("X")
            # ---- X = x @ Wih^T + bias : [128, 8, NS] ----
            X_sb = bpool.tile([128, 8, NS], F32, tag="X")
            pe_observe(tT_sb[:, 0:1], tT_sb[:, 0:1])
            xhist = []
            for n0 in range(0, NS, 512):
                nlen = min(512, NS - n0)
                pe_observe(xT16[:, 0, n0:n0 + 1], xT16[:, 1, n0 + nlen - 1:n0 + nlen])
                for m in range(8):
                    ps = psxpool.tile([128, 512], F32, tag="psx")
                    if len(xhist) >= 2:
                        pm, pn = xhist[-2]
                        pe_observe(tT_sb[:, 0:1], X_sb[:, pm, pn:pn + 1])
                    xhist.append((m, n0))
                    step = 0
                    for wsb in (wih_hi_sb, wih_lo_sb):
                        for kt in range(2):
                            dep_mm(
                                ps[:, :nlen], wsb[:, kt, m, :],
                                xT16[:, kt, n0:n0 + nlen],
                                start=(step == 0), stop=(step == 3))
                            step += 1
                    nc.vector.tensor_scalar(
                        X_sb[:, m, n0:n0 + nlen], ps[:, :nlen],
                        bias_sb[:, m:m + 1], None, add)
            if debug:
                nc.sync.dma_start(x_dbg[:], X_sb[:].rearrange("p a b -> p (a b)"))

            mark("LSTM")
            # ---- LSTM ----
            h_all, c_st = [], []
            for g in range(C.G):
                hg = bpool.tile([128, 2, B, C.S + 1], FP16, tag=f"h{g}", name=f"h{g}")
                cg = bpool.tile([128, 2, B], F32, tag=f"c{g}", name=f"c{g}")
                h_all.append(hg)
                c_st.append(cg)
            for g in range(C.G):
                nc.vector.memset(h_all[g][:, :, :, 0], 0.0)
                nc.vector.memset(c_st[g][:], 0.0)

            half = C.NPC // C.G
            for k in range(C.S):
                for g in range(C.G):
                    if k == C.W and g == 0:
                        hsl = h_all[0][:, :, 0, C.W]
                        nc.vector.tensor_tensor(hsl, hsl, hmask_sb[:], op=mult)
                        nc.vector.tensor_tensor(hsl, hsl, hinj_sb[:], op=add)
                        csl = c_st[0][:, :, 0]
                        nc.vector.tensor_tensor(csl, csl, cmask_sb[:], op=mult)
                        nc.vector.tensor_tensor(csl, csl, cinj_sb[:], op=add)
                    ps = pspool.tile([128, 6 * B], F32, tag="psg")
                    psg2 = psepool.tile([128, 2 * B], F32, tag="ep")
                    pe_observe(hmask_sb[:, 0:1], h_all[g][:, 0, 0:1, k])
                    st = half * g + k + (32 - C.W)
                    # g-gates (m=6,7) first into their own bank: tanh(g) can
                    # run while the i/f/o matmuls still stream.
                    for m in (6, 7):
                        for kt in range(2):
                            dep_mm(
                                psg2[:, (m - 6) * B:(m - 5) * B],
                                whh_sb[:, kt, m, :],
                                h_all[g][:, kt, :, k],
                                start=(kt == 0), stop=(kt == 1))
                    ug = spool.tile([128, 2, B], F32, tag="ug")
                    xslg = X_sb[:, 6:8, st: st + C.L * (B - 1) + 1: C.L]
                    nc.vector.tensor_tensor(
                        ug[:], psg2[:].rearrange("p (a b) -> p a b", a=2), xslg, op=add)
                    gt = spool.tile([128, 2 * B], F32, tag="gt")
                    nc.scalar.activation(gt[:], ug[:].rearrange("p a b -> p (a b)"), TANH)
                    for m in range(6):
                        for kt in range(2):
                            dep_mm(
                                ps[:, m * B:(m + 1) * B],
                                whh_sb[:, kt, m, :],
                                h_all[g][:, kt, :, k],
                                start=(kt == 0), stop=(kt == 1))
                    u = spool.tile([128, 6, B], F32, tag="u")
                    xsl = X_sb[:, 0:6, st: st + C.L * (B - 1) + 1: C.L]
                    nc.vector.tensor_tensor(
                        u[:], ps[:].rearrange("p (a b) -> p a b", a=6), xsl, op=add)
                    uf = u[:].rearrange("p a b -> p (a b)")
                    a = spool.tile([128, 6 * B], F32, tag="a")
                    nc.scalar.activation(a[:], uf[:, :6 * B], SIG)
                    cf = c_st[g][:].rearrange("p a b -> p (a b)")
                    t1 = spool.tile([128, 2 * B], F32, tag="t1")
                    nc.vector.tensor_tensor(t1[:], a[:, :2 * B], gt[:], op=mult)
                    nc.vector.tensor_tensor(cf, a[:, 2 * B:4 * B], cf, op=mult)
                    nc.vector.tensor_tensor(cf, cf, t1[:], op=add)
                    tc2 = spool.tile([128, 2 * B], F32, tag="tc")
                    nc.scalar.activation(tc2[:], cf, TANH)
                    nc.vector.tensor_tensor(
                        h_all[g][:, :, :, k + 1].rearrange("p a b -> p (a b)"),
                        a[:, 4 * B:6 * B], tc2[:], op=mult)
            if debug:
                nc.sync.dma_start(h_dbg[:], h_all[0][:].rearrange("p a b c -> p (a b c)"))

            mark("EMIS")
            # ---- emissions -> emis_sb [NTAGS, NS] ----
            emis_sb = bpool.tile([NTAGS, NS], FP16, tag="emis")
            if C.W < 32:
                nc.vector.memset(emis_sb[:, 0:32 - C.W], 0.0)
            epw = psepool.tile([NTAGS, 512], F32, tag="ep")
            pe_observe(hmask_sb[:, 0:1], h_all[0][:, 0, 0:1, C.S])
            step = 0
            for hl in range(2):
                for kt in range(2):
                    dep_mm(epw[:, :C.W], wout_sb[:, kt, hl, :],
                           h_all[0][:, kt, 0, 1:C.W + 1],
                           start=(step == 0), stop=(step == 3))
                    step += 1
            nc.vector.tensor_scalar(emis_sb[:, 32 - C.W:32], epw[:, :C.W],
                                    bout_sb[:NTAGS, :], None, add)
            lastcol = 0
            for g in range(C.G):
                for n0 in range(0, half, 512):
                    nlen = min(512, half - n0)
                    nb = nlen // C.L
                    b0 = n0 // C.L
                    ep = psepool.tile([NTAGS, 512], F32, tag="ep")
                    pe_observe(hmask_sb[:NTAGS, 0:1], emis_sb[:, lastcol:lastcol + 1])
                    step = 0
                    for hl in range(2):
                        for kt in range(2):
                            hap = h_all[g][:, kt, b0:b0 + nb, C.W + 1:C.W + 1 + C.L]
                            dep_mm(ep[:, :nlen], wout_sb[:, kt, hl, :], hap,
                                             start=(step == 0), stop=(step == 3))
                            step += 1
                    nc.vector.tensor_scalar(
                        emis_sb[:, 32 + g * half + n0:32 + g * half + n0 + nlen],
                        ep[:, :nlen], bout_sb[:NTAGS, :], None, add)
                    lastcol = 32 + g * half + n0
            if debug:
                nc.sync.dma_start(emis_dbg[:], emis_sb[:])

            mark("GATHER")
            # ---- AllGather ----
            nc.sync.dma_start(emis_local[:], emis_sb[:])
            nc.gpsimd.collective_compute(
                "AllGather", mybir.AluOpType.bypass,
                replica_groups=[[0, 1, 2, 3, 4, 5, 6, 7]],
                ins=[emis_local[:].rearrange("a b -> (a b)")],
                outs=[emis_all[:].rearrange("a b -> (a b)")],
            )
            emis_full = bpool.tile([120, NS], FP16, tag="efull")
            nc.sync.dma_start(emis_full[:], emis_all[:])

            # ---- selector matmuls: pick this core's Ef / Eb row-blocks ----
            efsel = bpool.tile([NTAGS, NS], F32, tag="efsel")
            ebsel = bpool.tile([NTAGS, NS], F32, tag="ebsel")
            pe_observe(hmask_sb[:120, 0:1], emis_full[:, 0:1])
            shist = []
            for dst, scol in ((efsel, 0), (ebsel, NTAGS)):
                for n0 in range(0, NS, 512):
                    nlen = min(512, NS - n0)
                    ps = psxpool.tile([128, 512], F32, tag="psx")
                    if len(shist) >= 2:
                        pd, pn = shist[-2]
                        pe_observe(tT_sb[:NTAGS, 0:1], pd[:, pn:pn + 1])
                    shist.append((dst, n0))
                    dep_mm(ps[:NTAGS, :nlen],
                                     esel_sb[:, scol:scol + NTAGS],
                                     emis_full[:, n0:n0 + nlen],
                                     start=True, stop=True)
                    nc.vector.tensor_copy(dst[:, n0:n0 + nlen], ps[:NTAGS, :nlen])
            # third selector: previous bwd block, cols 32..63 (warmup Eb for
            # the (v2==0, s==0) stream); zero selector on other cores.
            eb2 = bpool.tile([NTAGS, 32], F32, tag="eb2")
            ps2 = psxpool.tile([128, 512], F32, tag="psx")
            pd, pn = shist[-2]
            pe_observe(tT_sb[:NTAGS, 0:1], pd[:, pn:pn + 1])
            dep_mm(ps2[:NTAGS, :32], esel_sb[:, 2 * NTAGS:],
                             emis_full[:, 32:64], start=True, stop=True)
            nc.vector.tensor_copy(eb2[:], ps2[:NTAGS, :32])

            mark("FEATS")
            # ---- feats assembly ----
            feats_arr = bpool.tile([128, C.VCOLS], F32, tag="feats")
            nc.vector.memset(feats_arr[:], 0.0)
            fstage = bpool.tile([NTAGS, C.VS, C.VSTEP], F32, tag="fstage")
            ebrev = ebsel[:, ::-1]
            for s in range(C.VS):
                dst = fstage[:, s, :]
                t0 = spool.tile([NTAGS, C.VSTEP], F32, tag="fb0")
                t1b = spool.tile([NTAGS, C.VSTEP], F32, tag="fb1")
                c0, c1 = C.VCH * s, half + C.VCH * s
                nc.vector.tensor_scalar(t0[:], efsel[:, c0:c0 + C.VSTEP],
                                        pmask_sb[:NTAGS, 0:1], None, mult)
                nc.vector.tensor_scalar(t1b[:], efsel[:, c1:c1 + C.VSTEP],
                                        pmask_sb[:NTAGS, 1:2], None, mult)
                nc.vector.tensor_tensor(dst, t0[:], t1b[:], op=add)
                if s >= 1:
                    e0 = spool.tile([NTAGS, C.VSTEP], F32, tag="fb0")
                    e1 = spool.tile([NTAGS, C.VSTEP], F32, tag="fb1")
                    nc.vector.tensor_scalar(e0[:], ebrev[:, c0 - 32:c0 + C.VCH],
                                            pmask_sb[:NTAGS, 0:1], None, mult)
                    nc.vector.tensor_scalar(e1[:], ebrev[:, c1 - 32:c1 + C.VCH],
                                            pmask_sb[:NTAGS, 1:2], None, mult)
                    nc.vector.tensor_tensor(e0[:], e0[:], e1[:], op=add)
                    nc.vector.tensor_tensor(dst, dst, e0[:], op=add)
                else:
                    e0 = spool.tile([NTAGS, C.VCH], F32, tag="fc0")
                    e1m = spool.tile([NTAGS, C.VCH], F32, tag="fc1")
                    nc.vector.tensor_scalar(e0[:], ebrev[:, c0:c0 + C.VCH],
                                            pmask_sb[:NTAGS, 0:1], None, mult)
                    nc.vector.tensor_scalar(e1m[:], ebrev[:, c1:c1 + C.VCH],
                                            pmask_sb[:NTAGS, 1:2], None, mult)
                    nc.vector.tensor_tensor(e0[:], e0[:], e1m[:], op=add)
                    dstm = fstage[:, s, C.VW:]
                    nc.vector.tensor_tensor(dstm, dstm, e0[:], op=add)
                    w1 = spool.tile([NTAGS, 32], F32, tag="wb1")
                    nc.vector.tensor_scalar(w1[:], ebrev[:, half - 32:half],
                                            pmask_sb[:NTAGS, 1:2], None, mult)
                    nc.vector.tensor_tensor(w1[:], w1[:], eb2[:, ::-1], op=add)
                    dstw = fstage[:, s, 0:32]
                    nc.vector.tensor_tensor(dstw, dstw, w1[:], op=add)
                nc.sync.dma_start(
                    feats_arr[32 * s:32 * s + NTAGS, 1:1 + C.VSTEP], fstage[:, s, :])
            nc.vector.tensor_scalar(
                feats_arr[:, 0:C.VW + 1], feats_arr[:, 0:C.VW + 1],
                fmask_sb[:], None, mult)

            mark("VITERBI")
            # ---- viterbi ----
            fvs = bpool.tile([128, C.VCOLS], F32, tag="fvs")
            nc.vector.memset(fvs[:, 0:1], 0.0)
            for k in range(C.VSTEP):
                if k == C.VW:
                    sl = fvs[:, k:k + 1]
                    nc.vector.tensor_tensor(sl, sl, vmask_sb[:], op=mult)
                    nc.vector.tensor_tensor(sl, sl, vinit_sb[:], op=add)
                vscr = spool.tile([128, 32], F32, tag="vscr")
                nc.vector.tensor_scalar(
                    vscr[:], tT_sb[:], fvs[:, k:k + 1], feats_arr[:, k:k + 1],
                    add, add)
                nc.vector.tensor_reduce(
                    fvs[:, k + 1:k + 2], vscr[:], op=mybir.AluOpType.max,
                    axis=mybir.AxisListType.X, apply_transpose=True)

            nc.sync.dma_start(fvs_out[:], fvs[:])
            nc.sync.dma_start(feats_out[:], feats_arr[:])

    nc.compile()
    return nc


# ----------------------------------------------------------------------------
# host prep / post
# ----------------------------------------------------------------------------

def _split16(W):
    hi = W.astype(np.float16)
    lo = (W.astype(np.float32) - hi.astype(np.float32)).astype(np.float16)
    return hi, lo


def prep_core_inputs(cfg, core, sentence, E, Wih, Whh, bih, bhh, W_out_half,
                     b_out, Tm, h0d, c0d):
    C = cfg
    T = C.T
    fwd = core < 4
    base = (core % 4) * C.NPC

    dpos = np.arange(C.NSLOT) + base - 32
    opos = dpos if fwd else (T - 1 - dpos)
    valid = (dpos >= 0) & (dpos < T)
    rows = np.zeros((C.NSLOT, D), np.float32)
    vi = np.where(valid)[0]
    rows[vi] = E[sentence[opos[vi]]]
    xT = rows.T.reshape(2, 128, C.NSLOT).transpose(1, 0, 2)

    def tiles(Wm):
        t = np.zeros((128, 2, 8, 128), np.float32)
        for kt in range(2):
            for m in range(8):
                t[:, kt, m, :] = Wm[m * 128:(m + 1) * 128, kt * 128:(kt + 1) * 128].T
        return t

    whh16 = tiles(Whh.astype(np.float16).astype(np.float32)).astype(np.float16)
    hi, lo = _split16(Wih)
    wih_hi16 = tiles(hi.astype(np.float32)).astype(np.float16)
    wih_lo16 = tiles(lo.astype(np.float32)).astype(np.float16)
    bias = (bih + bhh).astype(np.float32).reshape(8, 128).T.copy()

    wh, wl = _split16(W_out_half)
    wout = np.zeros((128, 2, 2, NTAGS), np.float16)
    for kt in range(2):
        wout[:, kt, 0, :] = wh[:, kt * 128:(kt + 1) * 128].T
        wout[:, kt, 1, :] = wl[:, kt * 128:(kt + 1) * 128].T

    bout = np.zeros((128, 1), np.float32)
    bout[:NTAGS, 0] = b_out.astype(np.float32) / 2.0

    tT = np.full((128, 32), NEG, np.float32)
    for s in range(4):
        tT[32 * s:32 * s + NTAGS, :NTAGS] = Tm.T

    inject = core in (0, 4)
    hmask = np.zeros((128, 2), np.float16) if inject else np.ones((128, 2), np.float16)
    cmask = np.zeros((128, 2), np.float32) if inject else np.ones((128, 2), np.float32)
    hinj = np.zeros((128, 2), np.float16)
    cinj = np.zeros((128, 2), np.float32)
    if inject:
        hinj[:, 0] = h0d[:128].astype(np.float16)
        hinj[:, 1] = h0d[128:].astype(np.float16)
        cinj[:, 0] = c0d[:128]
        cinj[:, 1] = c0d[128:]

    vmask = np.ones((128, 1), np.float32)
    vinit = np.zeros((128, 1), np.float32)
    fmask = np.ones((128, 1), np.float32)
    if core == 0:
        vmask[0:32] = 0.0
        vinit[0:32] = NEG
        vinit[START_IX] = 0.0
        fmask[0:32] = 0.0

    vh, v2 = core // 2, core % 2
    esel = np.zeros((120, 3 * NTAGS), np.float16)
    for tag in range(NTAGS):
        esel[vh * NTAGS + tag, tag] = 1.0
        esel[(7 - vh) * NTAGS + tag, NTAGS + tag] = 1.0
        if v2 == 0 and vh >= 1:
            esel[(8 - vh) * NTAGS + tag, 2 * NTAGS + tag] = 1.0
    pmask = np.zeros((128, 2), np.float32)
    pmask[:, 0] = 1.0 if v2 == 0 else 0.0
    pmask[:, 1] = 1.0 - pmask[:, 0]

    return {
        "xT_in": np.ascontiguousarray(xT.reshape(128, 2 * C.NSLOT), np.float32),
        "whh_t": np.ascontiguousarray(whh16.reshape(128, 2048)),
        "wih_hi": np.ascontiguousarray(wih_hi16.reshape(128, 2048)),
        "wih_lo": np.ascontiguousarray(wih_lo16.reshape(128, 2048)),
        "bias_in": bias,
        "wout_in": np.ascontiguousarray(wout.reshape(128, 4 * NTAGS)),
        "bout_in": bout,
        "tT_in": tT,
        "hmask_in": hmask, "hinj_in": hinj, "cmask_in": cmask, "cinj_in": cinj,
        "vmask_in": vmask, "vinit_in": vinit, "fmask_in": fmask,
        "esel_in": esel, "pmask_in": pmask,
    }


def host_finish(cfg, fvs_list, feats_list, Tm):
    C = cfg
    T = C.T
    NCH = T // C.VCH
    fv_main = np.zeros((T, NTAGS), np.float32)
    fv_warm_end = np.zeros((NCH, NTAGS), np.float32)
    feats = np.zeros((T, NTAGS), np.float32)
    for m in range(NCH):
        v, s = m // C.VS, m % C.VS
        blk = slice(32 * s, 32 * s + NTAGS)
        fv_main[m * C.VCH:(m + 1) * C.VCH] = fvs_list[v][blk, 1 + C.VW:].T
        fv_warm_end[m] = fvs_list[v][blk, C.VW]
        feats[m * C.VCH:(m + 1) * C.VCH] = feats_list[v][blk, 1 + C.VW:].T

    delta = np.float64(0.0)
    for m in range(1, NCH):
        p = m * C.VCH - 1
        a = fv_main[p]
        tag = int(np.argmax(a))
        delta = (np.float64(a[tag]) + delta) - np.float64(fv_warm_end[m][tag])
    fv_incl_last = fv_main[T - 1] + feats[T - 1]
    score = np.float32((fv_incl_last.astype(np.float64) + delta + Tm[STOP_IX]).max())

    path = np.zeros(T, np.int32)
    cur = int(np.argmax(fv_incl_last + Tm[STOP_IX]))
    path[T - 1] = cur
    for p in range(T - 1, 0, -1):
        cur = int(np.argmax(fv_main[p - 1] + feats[p - 1] + Tm[cur]))
        path[p - 1] = cur
    return score, path


def build_in_maps(cfg, sentence, E, W_ih_f, W_hh_f, b_ih_f, b_hh_f, W_ih_b,
                  W_hh_b, b_ih_b, b_hh_b, W_out, b_out, transitions, h0, c0):
    sentence = np.asarray(sentence).astype(np.int64)
    E = np.asarray(E, np.float32)
    Tm = np.asarray(transitions, np.float32)
    W_out = np.asarray(W_out, np.float32)
    b_out = np.asarray(b_out, np.float32)
    h0 = np.asarray(h0, np.float32)
    c0 = np.asarray(c0, np.float32)
    perm = np.r_[0:256, 256:512, 768:1024, 512:768]  # i,f,o,g row order

    in_maps = []
    for core in range(8):
        fwd = core < 4
        Wih = np.asarray(W_ih_f if fwd else W_ih_b, np.float32)[perm]
        Whh = np.asarray(W_hh_f if fwd else W_hh_b, np.float32)[perm]
        bih = np.asarray(b_ih_f if fwd else b_ih_b, np.float32)[perm]
        bhh = np.asarray(b_hh_f if fwd else b_hh_b, np.float32)[perm]
        Wh = W_out[:, :HH] if fwd else W_out[:, HH:]
        h0d = h0[0] if fwd else h0[1]
        c0d = c0[0] if fwd else c0[1]
        in_maps.append(prep_core_inputs(
            cfg, core, sentence, E, Wih, Whh, bih, bhh, Wh, b_out, Tm, h0d, c0d))
    return in_maps, Tm


_PROGRAM_CACHE = {}


def kernel(sentence, E, W_ih_f, W_hh_f, b_ih_f, b_hh_f, W_ih_b, W_hh_b,
           b_ih_b, b_hh_b, W_out, b_out, transitions, h0, c0):
    from concourse import bass_utils

    cfg = CFG()
    in_maps, Tm = build_in_maps(
        cfg, sentence, E, W_ih_f, W_hh_f, b_ih_f, b_hh_f, W_ih_b, W_hh_b,
        b_ih_b, b_hh_b, W_out, b_out, transitions, h0, c0)

    key = (cfg.T, cfg.B)
    if key not in _PROGRAM_CACHE:
        _PROGRAM_CACHE[key] = build_program(cfg)
    nc = _PROGRAM_CACHE[key]

    res = bass_utils.run_bass_kernel_spmd(nc, in_maps, core_ids=list(range(8)))
    fvs_list = [np.asarray(res.results[c]["fvs_out"]) for c in range(8)]
    feats_list = [np.asarray(res.results[c]["feats_out"]) for c in range(8)]
    score, path = host_finish(cfg, fvs_list, feats_list, Tm)
    return score, path
